# revision 25
# baseline (speedup 1.0000x reference)
"""3x3 valid cross-correlation (single channel) + bias on 8 NeuronCores.

Strategy (data-parallel, hint-compliant): split the 8192-row image into 8
horizontal bands host-side with a (kh-1)=2-row halo, one band per core; the
weight/bias are replicated. Each core computes its [1024, 8190] output band.

I/O precision: the 2e-2 rel-err budget admits fp16 end-to-end data movement
(input quantization error ~1e-3 of output absmax), which halves HBM traffic
vs fp32: per core ~17MB in + ~16.8MB out ~= 94us at 358 GB/s. Host casts
x -> fp16 and upcasts the fp16 output back to fp32 (host time is not part of
HW exec time).

Device kernel: the vertical taps are folded into a banded stationary matrix
B_dj [128, 126] with B_dj[m+di, m] = w[di, dj]; a TensorE matmul
out = B_dj.T @ x_rows then computes, for each of 126 output rows, the 3-row
vertical reduction. The 3 horizontal taps become 3 fp16 matmuls accumulating
into the same PSUM bank, with the rhs slice shifted by dj columns. PSUM ->
SBUF copy adds the bias and downcasts to fp16 (ACT/DVE alternating), then
HWDGE DMA out. Loads ride the SP HWDGE ring, stores the ACT ring.
"""
import numpy as np

import concourse.mybir as mybir
from concourse.bacc import Bacc
from concourse import tile
from concourse.bass_utils import run_bass_kernel_spmd

H = W = 8192
KH = KW = 3
OH, OW = H - KH + 1, W - KW + 1          # 8190 x 8190
NCORES = 8
BAND = 1024                               # output rows per core (core 7 keeps 1022)
IN_ROWS = BAND + KH - 1                   # 1026 input rows per band
M_TILE = 126                              # output rows per strip (128 - KH + 1)
K_TILE = 128
N_TILE = 512                              # psum bank width in fp32
IO_DT = mybir.dt.float16
IO_NP = np.float16
MM_DT = mybir.dt.float16

_CACHE = {}

# Tail strip (16 output rows) packing: 7 column-blocks of 1170 output cols
# stacked across partitions (7 blocks x 18 input rows = 126 partitions), so the
# tail costs ~3.5k PE stream cycles instead of 24.6k (full-width streams).
NB = 7                                    # column blocks
RB = IN_ROWS - (8 * M_TILE)               # 18 input rows in tail
MB = BAND - (8 * M_TILE)                  # 16 output rows in tail
CB = OW // NB                             # 1170 output cols per block
MR = NB * MB                              # 112 psum partitions


def _col_tiles():
    tiles = []
    c0 = 0
    while c0 < OW:
        tiles.append((c0, min(N_TILE, OW - c0)))
        c0 += N_TILE
    return tiles


def _build(x_bufs=4, y_bufs=3, ps_bufs=8, x_chunks=2, y_chunks=2, warmup=48,
           pack_tail=True):
    nc = Bacc()
    xb = nc.dram_tensor("xb", [IN_ROWS, W], IO_DT, kind="ExternalInput")
    bands = nc.dram_tensor("bands", [K_TILE, KW * M_TILE], MM_DT,
                           kind="ExternalInput")
    bandsr = nc.dram_tensor("bandsr", [NB * RB, KW * MR], MM_DT,
                            kind="ExternalInput")
    biasb = nc.dram_tensor("biasb", [K_TILE, 1], mybir.dt.float32,
                           kind="ExternalInput")
    yb = nc.dram_tensor("yb", [BAND, OW], IO_DT, kind="ExternalOutput")
    ybr = nc.dram_tensor("ybr", [MR, CB], IO_DT, kind="ExternalOutput")

    n_strips = (BAND + M_TILE - 1) // M_TILE  # 9 (last strip: 16 rows)
    if pack_tail:
        n_strips -= 1
    ctiles = _col_tiles()

    with tile.TileContext(nc) as tc:
        with (
            tc.tile_pool(name="consts", bufs=1) as cpool,
            tc.tile_pool(name="xin", bufs=x_bufs) as xpool,
            tc.tile_pool(name="yout", bufs=y_bufs) as ypool,
            tc.tile_pool(name="pp", bufs=ps_bufs, space="PSUM") as pp,
        ):
            bands_t = cpool.tile([K_TILE, KW * M_TILE], MM_DT, name="bands_t")
            nc.sync.dma_start(out=bands_t, in_=bands.ap())
            bias_t = cpool.tile([K_TILE, 1], mybir.dt.float32, name="bias_t")
            nc.sync.dma_start(out=bias_t, in_=biasb.ap())

            if warmup:
                # Dense dummy matmuls during the first strip load: keeps PE's
                # HAM activity window busy so the clock is at 2.4 GHz by the
                # time real matmuls arrive (saves ~10us of cold-clock time).
                wt = cpool.tile([K_TILE, K_TILE], MM_DT, name="wt")
                nc.vector.memset(wt, 0)
                psw = pp.tile([K_TILE, N_TILE], mybir.dt.float32, name="psw",
                              tag="ps")
                for _ in range(warmup):
                    nc.tensor.matmul(psw[:, :K_TILE], wt, wt, start=True,
                                     stop=True)

            def emit_strip(s):
                r0 = s * M_TILE
                in_rows = min(K_TILE, IN_ROWS - r0)
                out_rows = min(M_TILE, BAND - r0)
                xt = xpool.tile([K_TILE, W], MM_DT, name="xt", tag="xt")
                xch = 4 if s == 0 else x_chunks  # finer first strip: earlier MM start
                xc = W // xch
                for k in range(xch):
                    lo, hi = k * xc, min(W, (k + 1) * xc)
                    nc.sync.dma_start(out=xt[:in_rows, lo:hi],
                                      in_=xb.ap()[r0:r0 + in_rows, lo:hi])
                yt = ypool.tile([M_TILE, OW], IO_DT, name="yt", tag="yt")
                for j, (c0, wdt) in enumerate(ctiles):
                    ps = pp.tile([M_TILE, N_TILE], mybir.dt.float32, name="ps",
                                 tag="ps")
                    for dj in range(KW):
                        nc.tensor.matmul(
                            ps[:out_rows, :wdt],
                            bands_t[:in_rows, dj * M_TILE: dj * M_TILE + out_rows],
                            xt[:in_rows, c0 + dj: c0 + dj + wdt],
                            start=(dj == 0),
                            stop=(dj == KW - 1),
                        )
                    if j % 2 == 0:
                        nc.scalar.add(yt[:out_rows, c0:c0 + wdt], ps[:out_rows, :wdt],
                                      bias_t[:out_rows, :])
                    else:
                        nc.vector.tensor_scalar_add(yt[:out_rows, c0:c0 + wdt],
                                                    ps[:out_rows, :wdt],
                                                    bias_t[:out_rows, :])
                yc = OW // y_chunks
                for k in range(y_chunks):
                    lo, hi = k * yc, (min(OW, (k + 1) * yc) if k + 1 < y_chunks else OW)
                    nc.scalar.dma_start(
                        out=yb.ap()[r0:r0 + out_rows, lo:hi], in_=yt[:out_rows, lo:hi])

            for s in range(n_strips):
                emit_strip(s)

            if pack_tail:
                # Tail strip: output rows 1008..1023 via 7 partition-packed
                # column blocks. xr[18b + r, c] = xb[1008 + r, 1170b + c].
                r0 = 8 * M_TILE
                bandsr_t = cpool.tile([NB * RB, KW * MR], MM_DT, name="bandsr_t")
                nc.sync.dma_start(out=bandsr_t, in_=bandsr.ap())
                xr = cpool.tile([NB * RB, CB + KW - 1], MM_DT, name="xr")
                for b in range(NB):
                    nc.sync.dma_start(
                        out=xr[b * RB:(b + 1) * RB, :],
                        in_=xb.ap()[r0:r0 + RB, b * CB: b * CB + CB + KW - 1])
                yr = cpool.tile([MR, CB], IO_DT, name="yr")
                c0 = 0
                j = 0
                while c0 < CB:
                    wdt = min(N_TILE, CB - c0)
                    ps = pp.tile([M_TILE, N_TILE], mybir.dt.float32, name="ps",
                                 tag="ps")
                    for dj in range(KW):
                        nc.tensor.matmul(
                            ps[:MR, :wdt],
                            bandsr_t[:, dj * MR: (dj + 1) * MR],
                            xr[:, c0 + dj: c0 + dj + wdt],
                            start=(dj == 0),
                            stop=(dj == KW - 1),
                        )
                    if j % 2 == 0:
                        nc.scalar.add(yr[:, c0:c0 + wdt], ps[:MR, :wdt],
                                      bias_t[:MR, :])
                    else:
                        nc.vector.tensor_scalar_add(yr[:, c0:c0 + wdt],
                                                    ps[:MR, :wdt],
                                                    bias_t[:MR, :])
                    c0 += wdt
                    j += 1
                nc.scalar.dma_start(out=ybr.ap(), in_=yr)
    nc.finalize()
    return nc


def _make_bands(weight: np.ndarray) -> np.ndarray:
    bands = np.zeros((K_TILE, KW * M_TILE), np.float32)
    m = np.arange(M_TILE)
    for dj in range(KW):
        for di in range(KH):
            bands[m + di, dj * M_TILE + m] = weight[di, dj]
    return bands


def _make_bandsr(weight: np.ndarray) -> np.ndarray:
    bandsr = np.zeros((NB * RB, KW * MR), np.float32)
    m = np.arange(MB)
    for b in range(NB):
        for dj in range(KW):
            for di in range(KH):
                bandsr[b * RB + m + di, dj * MR + b * MB + m] = weight[di, dj]
    return bandsr


def _run(inputs: dict, trace: bool = False):
    x = np.asarray(inputs["x"], dtype=np.float32)
    weight = np.asarray(inputs["weight"], dtype=np.float32)
    bias = np.asarray(inputs["bias"], dtype=np.float32)

    if "nc" not in _CACHE:
        _CACHE["nc"] = _build()
    nc = _CACHE["nc"]

    x_pad = np.zeros((NCORES * BAND + KH - 1, W), IO_NP)
    x_pad[:H] = x.astype(IO_NP)
    bands = _make_bands(weight).astype(IO_NP)
    bandsr = _make_bandsr(weight).astype(IO_NP)
    biasb = np.full((K_TILE, 1), bias[0], np.float32)

    in_maps = []
    for c in range(NCORES):
        r0 = c * BAND
        in_maps.append({
            "xb": np.ascontiguousarray(x_pad[r0:r0 + IN_ROWS]),
            "bands": bands,
            "bandsr": bandsr,
            "biasb": biasb,
        })

    res = run_bass_kernel_spmd(nc, in_maps, core_ids=list(range(NCORES)),
                               trace=trace)

    out = np.empty((OH, OW), np.float32)
    body = 8 * M_TILE
    for c in range(NCORES):
        r0 = c * BAND
        take = min(BAND, OH - r0)
        out[r0:r0 + min(take, body)] = res.results[c]["yb"][:min(take, body)]
        if take > body:
            ybr = np.asarray(res.results[c]["ybr"])  # [NB*MB, CB]
            tail = ybr.reshape(NB, MB, CB).transpose(1, 0, 2).reshape(MB, OW)
            out[r0 + body:r0 + take] = tail[:take - body]
    return out, res


def kernel(**inputs) -> np.ndarray:
    out, _ = _run(inputs, trace=False)
    return out


# revision 26
# speedup vs baseline: 1.0400x; 1.0400x over previous
"""3x3 valid cross-correlation (single channel) + bias on 8 NeuronCores.

Strategy (data-parallel, hint-compliant): split the 8192-row image into 8
horizontal bands host-side with a (kh-1)=2-row halo, one band per core; the
weight/bias are replicated. Each core computes its [1024, 8190] output band.

I/O precision: the 2e-2 rel-err budget admits fp16 end-to-end data movement
(input quantization error ~1e-3 of output absmax), which halves HBM traffic
vs fp32: per core ~17MB in + ~16.8MB out ~= 94us at 358 GB/s. Host casts
x -> fp16 and upcasts the fp16 output back to fp32 (host time is not part of
HW exec time).

Device kernel: the vertical taps are folded into a banded stationary matrix
B_dj [128, 126] with B_dj[m+di, m] = w[di, dj]; a TensorE matmul
out = B_dj.T @ x_rows then computes, for each of 126 output rows, the 3-row
vertical reduction. The 3 horizontal taps become 3 fp16 matmuls accumulating
into the same PSUM bank, with the rhs slice shifted by dj columns. PSUM ->
SBUF copy adds the bias and downcasts to fp16 (ACT/DVE alternating), then
HWDGE DMA out. Loads ride the SP HWDGE ring, stores the ACT ring.
"""
import numpy as np

import concourse.mybir as mybir
from concourse.bacc import Bacc
from concourse import tile
from concourse.bass_utils import run_bass_kernel_spmd

H = W = 8192
KH = KW = 3
OH, OW = H - KH + 1, W - KW + 1          # 8190 x 8190
NCORES = 8
BAND = 1024                               # output rows per core (core 7 keeps 1022)
IN_ROWS = BAND + KH - 1                   # 1026 input rows per band
M_TILE = 126                              # output rows per strip (128 - KH + 1)
K_TILE = 128
N_TILE = 512                              # psum bank width in fp32
IO_DT = mybir.dt.float16
IO_NP = np.float16
MM_DT = mybir.dt.float16

_CACHE = {}

# Tail strip (16 output rows) packing: 7 column-blocks of 1170 output cols
# stacked across partitions (7 blocks x 18 input rows = 126 partitions), so the
# tail costs ~3.5k PE stream cycles instead of 24.6k (full-width streams).
NB = 7                                    # column blocks
RB = IN_ROWS - (8 * M_TILE)               # 18 input rows in tail
MB = BAND - (8 * M_TILE)                  # 16 output rows in tail
CB = OW // NB                             # 1170 output cols per block
MR = NB * MB                              # 112 psum partitions


def _col_tiles():
    tiles = []
    c0 = 0
    while c0 < OW:
        tiles.append((c0, min(N_TILE, OW - c0)))
        c0 += N_TILE
    return tiles


def _build(x_bufs=3, y_bufs=3, ps_bufs=8, x_chunks=2, y_chunks=2, warmup=40,
           pack_tail=True):
    nc = Bacc()
    xb = nc.dram_tensor("xb", [IN_ROWS, W], IO_DT, kind="ExternalInput")
    bands = nc.dram_tensor("bands", [K_TILE, KW * M_TILE], MM_DT,
                           kind="ExternalInput")
    bandsr = nc.dram_tensor("bandsr", [NB * RB, KW * MR], MM_DT,
                            kind="ExternalInput")
    biasb = nc.dram_tensor("biasb", [K_TILE, 1], mybir.dt.float32,
                           kind="ExternalInput")
    yb = nc.dram_tensor("yb", [BAND, OW], IO_DT, kind="ExternalOutput")
    ybr = nc.dram_tensor("ybr", [MR, CB], IO_DT, kind="ExternalOutput")

    n_strips = (BAND + M_TILE - 1) // M_TILE  # 9 (last strip: 16 rows)
    if pack_tail:
        n_strips -= 1
    ctiles = _col_tiles()

    with tile.TileContext(nc) as tc:
        with (
            tc.tile_pool(name="consts", bufs=1) as cpool,
            tc.tile_pool(name="xin", bufs=x_bufs) as xpool,
            tc.tile_pool(name="yout", bufs=y_bufs) as ypool,
            tc.tile_pool(name="pp", bufs=ps_bufs, space="PSUM") as pp,
        ):
            bands_t = cpool.tile([K_TILE, KW * M_TILE], MM_DT, name="bands_t")
            nc.sync.dma_start(out=bands_t, in_=bands.ap())
            bias_t = cpool.tile([K_TILE, 1], mybir.dt.float32, name="bias_t")
            nc.sync.dma_start(out=bias_t, in_=biasb.ap())

            if warmup:
                # Dense dummy matmuls during the first strip load: keeps PE's
                # HAM activity window busy so the clock is at 2.4 GHz by the
                # time real matmuls arrive (saves ~10us of cold-clock time).
                wt = cpool.tile([K_TILE, K_TILE], MM_DT, name="wt")
                nc.vector.memset(wt, 0)
                psw = pp.tile([K_TILE, N_TILE], mybir.dt.float32, name="psw",
                              tag="ps")
                for _ in range(warmup):
                    nc.tensor.matmul(psw[:, :K_TILE], wt, wt, start=True,
                                     stop=True)

            def emit_strip(s):
                r0 = s * M_TILE
                in_rows = min(K_TILE, IN_ROWS - r0)
                out_rows = min(M_TILE, BAND - r0)
                xt = xpool.tile([K_TILE, W], MM_DT, name="xt", tag="xt")
                xch = 4 if s == 0 else x_chunks  # finer first strip: earlier MM start
                xc = W // xch
                for k in range(xch):
                    lo, hi = k * xc, min(W, (k + 1) * xc)
                    nc.sync.dma_start(out=xt[:in_rows, lo:hi],
                                      in_=xb.ap()[r0:r0 + in_rows, lo:hi])
                yt = ypool.tile([M_TILE, OW], IO_DT, name="yt", tag="yt")
                for j, (c0, wdt) in enumerate(ctiles):
                    ps = pp.tile([M_TILE, N_TILE], mybir.dt.float32, name="ps",
                                 tag="ps")
                    for dj in range(KW):
                        nc.tensor.matmul(
                            ps[:out_rows, :wdt],
                            bands_t[:in_rows, dj * M_TILE: dj * M_TILE + out_rows],
                            xt[:in_rows, c0 + dj: c0 + dj + wdt],
                            start=(dj == 0),
                            stop=(dj == KW - 1),
                        )
                    if j % 2 == 0:
                        nc.scalar.add(yt[:out_rows, c0:c0 + wdt], ps[:out_rows, :wdt],
                                      bias_t[:out_rows, :])
                    else:
                        nc.vector.tensor_scalar_add(yt[:out_rows, c0:c0 + wdt],
                                                    ps[:out_rows, :wdt],
                                                    bias_t[:out_rows, :])
                yc = OW // y_chunks
                for k in range(y_chunks):
                    lo, hi = k * yc, (min(OW, (k + 1) * yc) if k + 1 < y_chunks else OW)
                    nc.scalar.dma_start(
                        out=yb.ap()[r0:r0 + out_rows, lo:hi], in_=yt[:out_rows, lo:hi])

            for s in range(n_strips):
                emit_strip(s)

            if pack_tail:
                # Tail strip: output rows 1008..1023 via 7 partition-packed
                # column blocks. xr[18b + r, c] = xb[1008 + r, 1170b + c].
                r0 = 8 * M_TILE
                bandsr_t = cpool.tile([NB * RB, KW * MR], MM_DT, name="bandsr_t")
                nc.sync.dma_start(out=bandsr_t, in_=bandsr.ap())
                xr = cpool.tile([NB * RB, CB + KW - 1], MM_DT, name="xr")
                for b in range(NB):
                    nc.sync.dma_start(
                        out=xr[b * RB:(b + 1) * RB, :],
                        in_=xb.ap()[r0:r0 + RB, b * CB: b * CB + CB + KW - 1])
                yr = cpool.tile([MR, CB], IO_DT, name="yr")
                c0 = 0
                j = 0
                while c0 < CB:
                    wdt = min(N_TILE, CB - c0)
                    ps = pp.tile([M_TILE, N_TILE], mybir.dt.float32, name="ps",
                                 tag="ps")
                    for dj in range(KW):
                        nc.tensor.matmul(
                            ps[:MR, :wdt],
                            bandsr_t[:, dj * MR: (dj + 1) * MR],
                            xr[:, c0 + dj: c0 + dj + wdt],
                            start=(dj == 0),
                            stop=(dj == KW - 1),
                        )
                    if j % 2 == 0:
                        nc.scalar.add(yr[:, c0:c0 + wdt], ps[:MR, :wdt],
                                      bias_t[:MR, :])
                    else:
                        nc.vector.tensor_scalar_add(yr[:, c0:c0 + wdt],
                                                    ps[:MR, :wdt],
                                                    bias_t[:MR, :])
                    c0 += wdt
                    j += 1
                nc.scalar.dma_start(out=ybr.ap(), in_=yr)
    nc.finalize()
    return nc


def _make_bands(weight: np.ndarray) -> np.ndarray:
    bands = np.zeros((K_TILE, KW * M_TILE), np.float32)
    m = np.arange(M_TILE)
    for dj in range(KW):
        for di in range(KH):
            bands[m + di, dj * M_TILE + m] = weight[di, dj]
    return bands


def _make_bandsr(weight: np.ndarray) -> np.ndarray:
    bandsr = np.zeros((NB * RB, KW * MR), np.float32)
    m = np.arange(MB)
    for b in range(NB):
        for dj in range(KW):
            for di in range(KH):
                bandsr[b * RB + m + di, dj * MR + b * MB + m] = weight[di, dj]
    return bandsr


def _run(inputs: dict, trace: bool = False):
    x = np.asarray(inputs["x"], dtype=np.float32)
    weight = np.asarray(inputs["weight"], dtype=np.float32)
    bias = np.asarray(inputs["bias"], dtype=np.float32)

    if "nc" not in _CACHE:
        _CACHE["nc"] = _build()
    nc = _CACHE["nc"]

    x_pad = np.zeros((NCORES * BAND + KH - 1, W), IO_NP)
    x_pad[:H] = x.astype(IO_NP)
    bands = _make_bands(weight).astype(IO_NP)
    bandsr = _make_bandsr(weight).astype(IO_NP)
    biasb = np.full((K_TILE, 1), bias[0], np.float32)

    in_maps = []
    for c in range(NCORES):
        r0 = c * BAND
        in_maps.append({
            "xb": np.ascontiguousarray(x_pad[r0:r0 + IN_ROWS]),
            "bands": bands,
            "bandsr": bandsr,
            "biasb": biasb,
        })

    res = run_bass_kernel_spmd(nc, in_maps, core_ids=list(range(NCORES)),
                               trace=trace)

    out = np.empty((OH, OW), np.float32)
    body = 8 * M_TILE
    for c in range(NCORES):
        r0 = c * BAND
        take = min(BAND, OH - r0)
        out[r0:r0 + min(take, body)] = res.results[c]["yb"][:min(take, body)]
        if take > body:
            ybr = np.asarray(res.results[c]["ybr"])  # [NB*MB, CB]
            tail = ybr.reshape(NB, MB, CB).transpose(1, 0, 2).reshape(MB, OW)
            out[r0 + body:r0 + take] = tail[:take - body]
    return out, res


def kernel(**inputs) -> np.ndarray:
    out, _ = _run(inputs, trace=False)
    return out
